# revision 41
# baseline (speedup 1.0000x reference)
"""Trainium2 Bass kernel for nn_CTRPredictor (gnn_message_passing).

score[e] = dot(normalize(x[src[e]]), normalize(x[dst[e]]))  for E edges.

Strategy (8 NeuronCores, SPMD, ZERO cross-core communication):
  - Nodes are cut into 4 quarters of 25000.  Each core is assigned a PAIR of
    quarters; the 6 distinct pairs cover all 12 ordered cross (qa,qb) edge
    classes and 2 duplicate-pair cores take the 4 diagonal classes:
      core0 {0,1} (cross 01+10)   core4 {0,3}          core6 {0,1} (diag 00+11)
      core1 {2,3}                 core5 {1,2}          core7 {2,3} (diag 22+33)
    Every core lands ~80000 edges (E/8) with no load imbalance.
  - Host stages each core's two quarters as a raw bf16 table of 50176 rows
    (25088 padded positions per quarter).  The device normalizes all 50176
    rows locally (ACT square, DVE reduce/scale, chunked+pipelined) and writes
    a normalized bf16 table to its own DRAM.  Redundant compute replaces the
    AllGather of the baseline (which serialized 200us on the CC cores).
  - dma_gather indices are int16 (<32768), so a gather call only reaches a
    32768-row window.  Two overlapping windows (base 0 and base 17920) cover
    the 50176-row table; rows 17920..32767 are reachable from both.  Edges
    are host-assigned to one of 4 (src-window, dst-window) wiring classes
    (8 calls x 2688 edges each, count-clamped) via a small feasibility LP
    done greedily.
  - Per call: dma_gather src rows + dst rows (256B bf16 rows, 4 SWDGE
    queues), DVE bf16 multiply + grouped reduce -> scores.  Host un-permutes.
"""

import numpy as np

N = 100000
D = 128
E = 640000
CORES = 8

Q = 25000                 # real rows per quarter
NPOS = 25088              # padded positions per quarter (= 7 chunks)
TROWS = 2 * NPOS          # 50176 table rows per core
CH = 14                   # normalize chunks
KCH = 28                  # rows per partition per chunk
RCH = 128 * KCH           # 3584 rows per chunk
W1 = 17920                # second gather window base (chunk 5)
WIN = 32768               # int16 gather window size

NCLS = 4                  # wiring classes: (w0,w0),(w0,w1),(w1,w0),(w1,w1)
CPC = 8                   # calls per class
NCALL = NCLS * CPC        # 32 gather call slots
GCALL = 2688              # edge slots per call
CCOL = GCALL // 128       # 21 score columns per call
ICOL = GCALL // 16        # 168 index columns per call
SCOL = NCALL * CCOL       # 672 score columns
CAP = CPC * GCALL         # 21504 per-class capacity
MINCNT = 128              # pad tiny calls up to this many gathered rows

# The table is TWO dram tensors (halves): half 0 = low 12500 rows of each
# quarter, half 1 = high halves; 25088 padded tokens each.  Tile-framework
# deps on DRAM tiles are whole-tensor, so splitting the table lets class
# (0,0) gathers start when half 0 is normalized, overlapping half 1.
HQ = 12500                # rows per quarter-half
HPAD = 12544              # padded tokens per quarter-half
NTOK = 2 * HPAD           # 25088 tokens per table half
HCH = 7                   # normalize chunks per half

# class -> (src_half, dst_half)
CLS_HALF = [(0, 0), (0, 1), (1, 0), (1, 1)]

# core -> (quarterA, quarterB); cores 6,7 duplicate pairs and take diagonals
CORE_QUARTERS = [(0, 1), (2, 3), (0, 2), (1, 3), (0, 3), (1, 2), (0, 1), (2, 3)]
_PAIR_CORE = {frozenset(p): i for i, p in enumerate(CORE_QUARTERS[:6])}

_CACHE = {}
LAST_RESULTS = None
RUN_KWARGS = {}  # extra kwargs for run_bass_kernel_spmd (used by test harness)


def _build():
    from concourse import bass, bacc, tile, mybir

    f32 = mybir.dt.float32
    bf16 = mybir.dt.bfloat16
    i16 = mybir.dt.int16
    i32 = mybir.dt.int32

    nc = bacc.Bacc("TRN2", target_bir_lowering=False, debug=False,
                   num_devices=CORES, num_swdge_queues=4,
                   dynamic_dma_scratch_size=40960)

    xraw_d = nc.dram_tensor("xraw", [128, CH * KCH * D], bf16,
                            kind="ExternalInput")
    sidx_d = nc.dram_tensor("src_idx", [128, NCALL * ICOL], i16,
                            kind="ExternalInput")
    didx_d = nc.dram_tensor("dst_idx", [128, NCALL * ICOL], i16,
                            kind="ExternalInput")
    cnt_d = nc.dram_tensor("cnt", [1, NCALL], i32, kind="ExternalInput")
    out_d = nc.dram_tensor("out", [128, SCOL], f32, kind="ExternalOutput")

    with tile.TileContext(nc) as tc:
        with tc.tile_pool(name="dram", bufs=1, space="DRAM") as dp, \
             tc.tile_pool(name="persist", bufs=1) as pp:

            # ---- index tables + score accumulator ----
            # NOTE: the tile framework assigns SWDGE completion-sem lanes
            # round-robin over Pool DMA instructions in program order; the
            # gathers cycle queues 0..3, so the number of earlier Pool DMAs
            # must stay ≡ 0 (mod 4) to keep lane<->queue pairing aligned.
            sidx = pp.tile([128, NCALL * ICOL], i16)
            didx = pp.tile([128, NCALL * ICOL], i16)
            cnt = pp.tile([1, NCALL], i32)
            nc.scalar.dma_start(out=sidx[:, :], in_=sidx_d.ap())
            nc.sync.dma_start(out=didx[:, :], in_=didx_d.ap())
            nc.sync.dma_start(out=cnt[:, :], in_=cnt_d.ap())
            score = pp.tile([128, SCOL], f32)
            ns = pp.tile([128, CH * KCH], f32)
            rns = pp.tile([128, CH * KCH], f32)

            # ---- normalized table halves in this core's DRAM ----
            tabs = [dp.tile([NTOK, D], bf16, name="tab0"),
                    dp.tile([NTOK, D], bf16, name="tab1")]

            # ---- phase 0: normalize the 50176-row local table, 14 chunks.
            # chunk j holds rows [RCH*j, RCH*(j+1)); partition p owns rows
            # RCH*j + KCH*p + k (k<KCH) so DRAM writes are 28*256B runs.
            # single-queue HWDGE fans out to only ~5 DMA engines (~110 GB/s);
            # round-robin chunk loads/stores over sync + act HWDGE queues and
            # the gpsimd SWDGE queue to engage more DMA engines.
            # NO Pool dma_starts at all: any Pool DMA (pinned to SWDGE queue
            # 0) permanently locks its round-robin sem lane to queue 0,
            # which then collides with the gathers cycling queues 0-3.
            sy, sc = nc.sync, nc.scalar
            ld_eng = [sy, sc] * 7
            st_eng = [sc, sy] * 7
            creg = nc.gpsimd.alloc_register("cnt_reg")
            with tc.tile_pool(name="ph0", bufs=3) as p0, \
                 tc.tile_pool(name="sqp", bufs=2) as sqp, \
                 tc.tile_pool(name="ga", bufs=5) as ga, \
                 tc.tile_pool(name="gb", bufs=5) as gb:

                def emit_chunk(j):
                    c0 = j * KCH
                    raw = p0.tile([128, KCH * D], bf16, tag="raw")
                    ld_eng[j].dma_start(
                        out=raw[:, :],
                        in_=xraw_d.ap()[:, c0 * D:(c0 + KCH) * D])
                    sq = sqp.tile([128, KCH * D], bf16, tag="sq")
                    nc.scalar.activation(
                        out=sq[:, :], in_=raw[:, :],
                        func=mybir.ActivationFunctionType.Square)
                    nc.vector.tensor_reduce(
                        out=ns[:, c0:c0 + KCH],
                        in_=sq[:, :].rearrange("p (r d) -> p r d", d=D),
                        axis=mybir.AxisListType.X,
                        op=mybir.AluOpType.add,
                    )
                    nc.scalar.activation(
                        out=ns[:, c0:c0 + KCH], in_=ns[:, c0:c0 + KCH],
                        func=mybir.ActivationFunctionType.Sqrt)
                    nc.vector.reciprocal(out=rns[:, c0:c0 + KCH],
                                         in_=ns[:, c0:c0 + KCH])
                    ntile = p0.tile([128, KCH * D], bf16, tag="ntile")
                    nc.vector.tensor_mul(
                        out=ntile[:, :].rearrange("p (r d) -> p r d", d=D),
                        in0=raw[:, :].rearrange("p (r d) -> p r d", d=D),
                        in1=rns[:, c0:c0 + KCH].unsqueeze(-1).to_broadcast(
                            [128, KCH, D]),
                    )
                    jh = j % HCH
                    st_eng[j].dma_start(
                        out=tabs[j // HCH][RCH * jh:RCH * (jh + 1), :]
                        .rearrange("(p k) d -> p (k d)", p=128),
                        in_=ntile[:, :],
                    )

                qn = 0

                def emit_call(cls, k):
                    nonlocal qn
                    call = cls * CPC + k
                    sh, dh = CLS_HALF[cls]
                    col0 = call * ICOL
                    xs_t = ga.tile([128, CCOL * D], bf16, tag="A")
                    xd_t = gb.tile([128, CCOL * D], bf16, tag="B")
                    nc.gpsimd.reg_load(creg, cnt[0:1, call:call + 1])
                    nc.gpsimd.dma_gather(
                        out_ap=xs_t[:, :].rearrange(
                            "p (c d) -> p c d", d=D),
                        in_ap=tabs[sh][:, :],
                        idxs_ap=sidx[:, col0:col0 + ICOL],
                        num_idxs=GCALL, num_idxs_reg=creg, elem_size=D,
                        single_packet=False, queue_num=qn % 4,
                    )
                    qn += 1
                    nc.gpsimd.dma_gather(
                        out_ap=xd_t[:, :].rearrange(
                            "p (c d) -> p c d", d=D),
                        in_ap=tabs[dh][:, :],
                        idxs_ap=didx[:, col0:col0 + ICOL],
                        num_idxs=GCALL, num_idxs_reg=creg, elem_size=D,
                        single_packet=False, queue_num=qn % 4,
                    )
                    qn += 1
                    nc.vector.tensor_mul(out=xs_t[:, :], in0=xs_t[:, :],
                                         in1=xd_t[:, :])
                    sc0 = call * CCOL
                    nc.vector.tensor_reduce(
                        out=score[:, sc0:sc0 + CCOL],
                        in_=xs_t[:, :].rearrange("p (c d) -> p c d", d=D),
                        axis=mybir.AxisListType.X,
                        op=mybir.AluOpType.add,
                    )

                # NOTE: interleaving gather calls between the two normalize
                # halves measured faster (526us) but is RACY on hardware:
                # DMAs sharing an HWDGE sem lane across the sync/scalar
                # queues can complete out of tick order, so a gather's
                # lane-count wait can be satisfied by the WRONG set of
                # completed DMAs and read unnormalized rows.  Keep the
                # normalize fully ahead of the gathers in program order.
                for j in range(CH):
                    emit_chunk(j)
                for k in range(CPC):
                    emit_call(0, k)
                for k in range(CPC):
                    for c in (1, 2, 3):
                        emit_call(c, k)

                nc.sync.dma_start(out=out_d.ap(), in_=score[:, :])

    nc.compile()
    return nc


def _wrap_idx(flat):
    """[GCALL] int16 -> [128, ICOL] in dma_gather's 16-partition wrap."""
    blk = flat.reshape(ICOL, 16).T  # index i at [i%16, i//16]
    return np.tile(blk, (8, 1))


def _prepare_core(core, s_half, s_tok, d_half, d_tok, edge_ids):
    """Slot one core's edges into the 32 gather calls (class = half pair).

    Returns (sidx, didx, counts, edge_ids_in_slot_order, rows, cols).
    """
    ne = s_tok.size
    cls = s_half * 2 + d_half
    loads = np.bincount(cls, minlength=NCLS)
    if loads.max() > CAP:
        raise ValueError(f"class overflow on core {core}: {loads}")

    sidx = np.zeros((128, NCALL * ICOL), dtype=np.int16)
    didx = np.zeros((128, NCALL * ICOL), dtype=np.int16)
    counts = np.zeros(NCALL, dtype=np.int32)
    rows = np.empty(ne, dtype=np.int64)
    cols = np.empty(ne, dtype=np.int64)
    ids_out = np.empty(ne, dtype=np.int64)
    pos = 0
    for c in range(NCLS):
        sel = np.flatnonzero(cls == c)
        # ascending src addresses give the src-side gather descriptors
        # HBM locality (the dst side stays random)
        sel = sel[np.argsort(s_tok[sel], kind="stable")]
        nsel_c = sel.size
        for k in range(CPC):
            call = c * CPC + k
            lo = min(k * GCALL, nsel_c)
            hi = min(lo + GCALL, nsel_c)
            ids = sel[lo:hi]
            nsel = ids.size
            s_pad = np.full(GCALL, -1, dtype=np.int16)
            d_pad = np.full(GCALL, -1, dtype=np.int16)
            s_pad[:nsel] = s_tok[ids]
            d_pad[:nsel] = d_tok[ids]
            ncnt = nsel
            if ncnt < MINCNT:
                # keep tiny/empty calls harmlessly busy on token 0
                s_pad[ncnt:MINCNT] = 0
                d_pad[ncnt:MINCNT] = 0
                ncnt = MINCNT
            col0 = call * ICOL
            sidx[:, col0:col0 + ICOL] = _wrap_idx(s_pad)
            didx[:, col0:col0 + ICOL] = _wrap_idx(d_pad)
            counts[call] = ncnt
            j = np.arange(nsel)
            rows[pos:pos + nsel] = j % 128
            cols[pos:pos + nsel] = call * CCOL + j // 128
            ids_out[pos:pos + nsel] = edge_ids[ids]
            pos += nsel
    assert pos == ne
    return sidx, didx, counts, ids_out, rows, cols


def kernel(x, src, dst):
    global LAST_RESULTS
    from concourse.bass_utils import run_bass_kernel_spmd

    if "nc" not in _CACHE:
        _CACHE["nc"] = _build()
    nc = _CACHE["nc"]

    x32 = np.asarray(x, dtype=np.float32)
    xbf = x32.astype(np.dtype("bfloat16")) if hasattr(np, "bfloat16") else None
    try:
        import ml_dtypes
        xbf = x32.astype(ml_dtypes.bfloat16)
    except ImportError:
        if xbf is None:
            raise
    src_i = np.asarray(src).astype(np.int64)
    dst_i = np.asarray(dst).astype(np.int64)

    qa_all = src_i // Q
    qb_all = dst_i // Q

    # edge -> core
    core_of = np.empty(E, dtype=np.int64)
    diag = qa_all == qb_all
    core_of[diag] = np.where(qa_all[diag] < 2, 6, 7)
    cross = ~diag
    lo = np.minimum(qa_all[cross], qb_all[cross])
    hi = np.maximum(qa_all[cross], qb_all[cross])
    pair_code = lo * 4 + hi  # (0,1)=1 (2,3)=11 (0,2)=2 (1,3)=7 (0,3)=3 (1,2)=6
    lut = np.full(16, -1, dtype=np.int64)
    for pair, corei in _PAIR_CORE.items():
        a, b = sorted(pair)
        lut[a * 4 + b] = corei
    core_of[cross] = lut[pair_code]

    in_maps = []
    inv = []
    for i in range(CORES):
        A, B = CORE_QUARTERS[i]
        ids = np.flatnonzero(core_of == i)
        s, d = src_i[ids], dst_i[ids]

        # node -> (half, token): half 0 = low HQ rows of each quarter
        def node_map(n):
            q = n // Q
            qi = np.where(q == A, 0, 1)
            local = n % Q
            half = (local >= HQ).astype(np.int64)
            tok = qi * HPAD + np.where(half == 0, local, local - HQ)
            return half, tok

        sq_, dq_ = s // Q, d // Q
        if not (((sq_ == A) | (sq_ == B)) & ((dq_ == A) | (dq_ == B))).all():
            raise AssertionError("edge routed to wrong core")
        sh, st_ = node_map(s)
        dh, dt_ = node_map(d)

        sidx, didx, counts, ids_slot, rows, cols = _prepare_core(
            i, sh, st_, dh, dt_, ids)
        inv.append((ids_slot, rows, cols))

        # raw halves; normalize-chunk row RCH*j + KCH*p + k of half h lives
        # at xraw[p, ((h*HCH + j)*KCH + k)*D : ...]
        xraw = np.empty((128, CH * KCH * D), dtype=xbf.dtype)
        for h in (0, 1):
            tok = np.empty((NTOK, D), dtype=xbf.dtype)
            tok[:] = xbf[0]  # pad rows, never gathered
            for qi, qq in enumerate((A, B)):
                base = qq * Q + (0 if h == 0 else HQ)
                tok[qi * HPAD:qi * HPAD + HQ] = xbf[base:base + HQ]
            xraw[:, h * HCH * KCH * D:(h + 1) * HCH * KCH * D] = (
                tok.reshape(HCH, 128, KCH, D).transpose(1, 0, 2, 3)
                .reshape(128, HCH * KCH * D))

        in_maps.append({
            "xraw": np.ascontiguousarray(xraw),
            "src_idx": np.ascontiguousarray(sidx),
            "dst_idx": np.ascontiguousarray(didx),
            "cnt": np.ascontiguousarray(counts.reshape(1, NCALL)),
        })

    res = run_bass_kernel_spmd(nc, in_maps, core_ids=list(range(CORES)),
                               **RUN_KWARGS)
    LAST_RESULTS = res

    out = np.empty(E, dtype=np.float32)
    for i in range(CORES):
        tilev = np.asarray(res.results[i]["out"])
        ids_slot, rows, cols = inv[i]
        out[ids_slot] = tilev[rows, cols]
    return out.reshape(E, 1)
